# revision 1
# baseline (speedup 1.0000x reference)
"""Trainium2 Bass kernel for the segment-reduce masked-CE loss (nn_NewLoss).

Reference math (N=64, C=46, P=2048, MP=256):
    assignment[n, p] = 1 + (p * MP) // P  (contiguous segments of 8 frames)
    pooled[n, q, c]  = mean over the 8 frames of segment q of input[n, c, :]
    loss = -sum_{n,q} lab_mask[n,q] * log_softmax(pooled)[n, q, target[n,q]]

Sharding: data-parallel over batch n across 8 cores (8 items per core);
each core returns a partial-loss scalar, summed on the host.

Per-core layout: the 368 local (item, channel) rows are packed into 3 slots
of 128 partitions (zero-padded to 384) with q=0..255 along the free dim:
    slot tile X_s[u, p],  row r = 128*s + u = 46*item + c
Pipeline per slot: DMA -> window-8 reduce (DVE) -> exp (ACT) -> PE item-sum
matmuls (sumexp + picked-class via an on-device one-hot built from a PE
broadcast of targets), then a short log/mask/reduce epilogue.
"""

import numpy as np

import concourse.bacc as bacc
import concourse.bass as bass
import concourse.tile as tile
from concourse import mybir
from concourse.bass_utils import run_bass_kernel_spmd

F32 = mybir.dt.float32
BF16 = mybir.dt.bfloat16

N, C, P, MP = 64, 46, 2048, 256
NCORES = 8
NLOC = N // NCORES            # 8 batch items per core
ROWS = NLOC * C               # 368 (item, channel) rows per core
SLOTS = (ROWS + 127) // 128   # 3 partition slots
W = P // MP                   # 8-frame pooling window

# aux column layout (fp32, [128, AUXW]):
#   [0:256)    rows 0..7: mask8 (lab_mask per (item, q))
#   [256:259)  c_col per slot: c(128s+u) = (128s+u) % 46 for valid rows, -1 pad
#   [259:260)  ones
OFF_MSK = 0
OFF_CCOL = MP
OFF_ONES = OFF_CCOL + SLOTS
AUXW = OFF_ONES + 1

# selb column layout (bf16, [128, SELW]) -- PE operands; 0/1 selectors are
# exact in bf16, so matmuls run single-pass:
#   [0:24)     isel:  isel[u, 8s+i] = 1 iff row 128s+u belongs to item i
#   [24:792)   ohp: per-slot masked one-hot, ohp[u, 256s+q] =
#              -1/8 if c(128s+u) == target[item(128s+u), q] else 0
OFF_B_ISEL = 0
OFF_B_OHP = NLOC * SLOTS
SELW = OFF_B_OHP + SLOTS * MP


def _build_nc():
    nc = bacc.Bacc("TRN2", target_bir_lowering=False)

    x_d = nc.dram_tensor("x", [128, SLOTS * P], BF16, kind="ExternalInput")
    aux_d = nc.dram_tensor("aux", [128, AUXW], F32, kind="ExternalInput")
    selb_d = nc.dram_tensor("selb", [128, SELW], BF16, kind="ExternalInput")
    loss_d = nc.dram_tensor("loss", [NLOC, 2], F32, kind="ExternalOutput")

    with tile.TileContext(nc) as tc:
        with (
            tc.tile_pool(name="xin", bufs=1) as xin,
            tc.tile_pool(name="pp", bufs=1) as pp,
            tc.tile_pool(name="psum", bufs=2, space="PSUM") as psum,
            tc.tile_pool(name="acc", bufs=1, space="PSUM") as accp,
            tc.tile_pool(name="small", bufs=1) as small,
        ):
            # x first on both HWDGE rings (sync + scalar) so the pool reduces
            # start as early as possible; aux/selb are only needed late, so
            # they queue after x.  The first chunk is small (512 cols) to cut
            # the DMA latency before the first reduce; chunk boundaries land
            # on 8-col windows so each reduce covers whole q ranges.
            chunk_cols = {
                0: [(0, 512), (512, 1280), (1280, 2048)],
                1: [(0, 1024), (1024, 2048)],
                2: [(0, 1024), (1024, 2048)],
            }
            # All x chunks go on the sync ring: HWDGE FIFO per ring means
            # they complete in consumption order, keeping the DVE reduce
            # pipeline gapless (a single DMA already spans all 16 SDMA
            # engines, so one ring gets full bandwidth).  Constants ride
            # the scalar ring in parallel.
            xs = []
            for s in range(SLOTS):
                xt = xin.tile([128, P], BF16, tag=f"x{s}")
                for c0, c1 in chunk_cols[s]:
                    nc.sync.dma_start(
                        out=xt[:, c0:c1],
                        in_=x_d[:, s * P + c0 : s * P + c1],
                    )
                xs.append(xt)

            aux_t = small.tile([128, AUXW], F32)
            nc.scalar.dma_start(out=aux_t[:], in_=aux_d[:])
            selb_t = small.tile([128, SELW], BF16)
            nc.scalar.dma_start(out=selb_t[:], in_=selb_d[:])
            msk8 = aux_t[0:NLOC, OFF_MSK : OFF_MSK + MP]
            ones8 = aux_t[0:NLOC, OFF_ONES : OFF_ONES + 1]

            s8_t = accp.tile([NLOC, MP], F32, tag="S8")
            px8_t = accp.tile([NLOC, MP], F32, tag="PX8")
            for s in range(SLOTS):
                isel_s = selb_t[:, OFF_B_ISEL + NLOC * s : OFF_B_ISEL + NLOC * (s + 1)]
                ohp_s = selb_t[:, OFF_B_OHP + MP * s : OFF_B_OHP + MP * (s + 1)]

                # Window-8 pool in two stages: a bf16 tensor_tensor fold
                # (8->4, runs at DVE 2x_1P mode: 16-bit + inner step 1) then
                # a 1x-mode reduce over the remaining 4 -- halves the reads
                # the 1x reduce has to stream.
                p_t = pp.tile([128, MP], F32, tag=f"p{s}")
                f_t = pp.tile([128, P // 2], BF16, tag=f"f{s}")
                for c0, c1 in chunk_cols[s]:
                    xv3 = xs[s][:, c0:c1].rearrange("u (q w) -> u q w", w=W)
                    fv3 = f_t[:, c0 // 2 : c1 // 2].rearrange(
                        "u (q w) -> u q w", w=W // 2
                    )
                    nc.vector.tensor_tensor(
                        fv3, xv3[:, :, 0 : W // 2], xv3[:, :, W // 2 : W],
                        mybir.AluOpType.add,
                    )
                    nc.vector.reduce_sum(
                        out=p_t[:, c0 // W : c1 // W],
                        in_=fv3,
                        axis=mybir.AxisListType.X,
                    )
                # sumexp: S8[i, q] += sum_u isel[u, i] * exp(pooled[u, q] / 8)
                xe_t = pp.tile([128, MP], BF16, tag=f"xe{s}")
                nc.scalar.activation(
                    out=xe_t[:],
                    in_=p_t[:],
                    func=mybir.ActivationFunctionType.Exp,
                    scale=1.0 / W,
                )
                nc.tensor.matmul(
                    out=s8_t[:],
                    lhsT=isel_s,
                    rhs=xe_t[:],
                    start=(s == 0),
                    stop=(s == SLOTS - 1),
                )
                # picked: M = ohp * pooled, summed per item by the PE.  The
                # last slot's multiply runs on DVE (free after its reduces)
                # to keep the tail off the slower GPSIMD path.
                m_t = pp.tile([128, MP], BF16, tag=f"m{s}")
                m_eng = nc.vector if s == SLOTS - 1 else nc.gpsimd
                m_eng.tensor_tensor(m_t[:], ohp_s, p_t[:], mybir.AluOpType.mult)
                nc.tensor.matmul(
                    out=px8_t[:],
                    lhsT=isel_s,
                    rhs=m_t[:],
                    start=(s == 0),
                    stop=(s == SLOTS - 1),
                )

            # The lab mask is folded into ohp on the host, so px8 is already
            # masked: reduce it as soon as it lands (overlaps the Ln table
            # load).  loss = sum_q msk*ln(S8) + sum_q px8, summed on host.
            cv_t = small.tile([NLOC, 2], F32)
            nc.vector.reduce_sum(
                out=cv_t[:, 1:2], in_=px8_t[:], axis=mybir.AxisListType.X
            )
            lse_t = small.tile([NLOC, MP], F32)
            nc.scalar.activation(
                out=lse_t[:], in_=s8_t[:], func=mybir.ActivationFunctionType.Ln
            )
            z_t = small.tile([NLOC, MP], F32)
            nc.vector.tensor_tensor(z_t[:], lse_t[:], msk8, mybir.AluOpType.mult)
            nc.vector.reduce_sum(
                out=cv_t[:, 0:1], in_=z_t[:], axis=mybir.AxisListType.X
            )
            nc.sync.dma_start(out=loss_d[:], in_=cv_t[:])

    nc.finalize()
    return nc


_NC = None


def _get_nc():
    global _NC
    if _NC is None:
        _NC = _build_nc()
    return _NC


def _make_aux():
    import ml_dtypes

    aux = np.zeros((128, AUXW), dtype=np.float32)
    selb = np.zeros((128, SELW), dtype=ml_dtypes.bfloat16)
    rows = np.arange(SLOTS * 128)
    item = np.minimum(rows // C, NLOC - 1)
    valid = rows < ROWS
    isel = np.zeros((SLOTS * 128, NLOC), dtype=np.float32)
    isel[valid, item[valid]] = 1.0
    isel = isel.reshape(SLOTS, 128, NLOC)
    for s in range(SLOTS):
        selb[:, OFF_B_ISEL + NLOC * s : OFF_B_ISEL + NLOC * (s + 1)] = isel[s]
    aux[:, OFF_ONES] = 1.0
    return aux, selb


def make_in_maps(input, target, lab_mask):
    import ml_dtypes

    inp = np.asarray(input)
    tgt = np.asarray(target)
    msk = np.asarray(lab_mask)
    aux_base, selb_base = _make_aux()
    in_maps = []
    for c in range(NCORES):
        xl = np.asarray(inp[c * NLOC : (c + 1) * NLOC], dtype=ml_dtypes.bfloat16)
        xl = xl.reshape(ROWS, P)
        xp = np.zeros((SLOTS * 128, P), dtype=ml_dtypes.bfloat16)
        xp[:ROWS] = xl
        xd = np.ascontiguousarray(
            xp.reshape(SLOTS, 128, P).transpose(1, 0, 2).reshape(128, SLOTS * P)
        )
        aux = aux_base.copy()
        aux[0:NLOC, OFF_MSK : OFF_MSK + MP] = msk[c * NLOC : (c + 1) * NLOC].astype(
            np.float32
        )
        selb = selb_base.copy()
        tl = tgt[c * NLOC : (c + 1) * NLOC]  # [8, 256] int
        rows = np.arange(SLOTS * 128)
        item = np.minimum(rows // C, NLOC - 1)
        cval = rows % C
        valid = rows < ROWS
        ml = msk[c * NLOC : (c + 1) * NLOC].astype(np.float32)  # [8, 256]
        ohp = (tl[item, :] == cval[:, None]) & valid[:, None]
        ohp = ohp.astype(np.float32) * (-1.0 / W) * ml[item, :]
        ohp = ohp.reshape(SLOTS, 128, MP)
        for s in range(SLOTS):
            selb[:, OFF_B_OHP + MP * s : OFF_B_OHP + MP * (s + 1)] = ohp[s].astype(
                ml_dtypes.bfloat16
            )
        in_maps.append({"x": xd, "aux": aux, "selb": selb})
    return in_maps


def kernel(input, target, assignment, lab_mask, _trace=False):
    in_maps = make_in_maps(input, target, lab_mask)
    nc = _get_nc()
    res = run_bass_kernel_spmd(nc, in_maps, core_ids=list(range(NCORES)), trace=_trace)
    total = np.float64(0.0)
    for r in res.results:
        total += np.float64(r["loss"].sum())
    out = np.array(total, dtype=np.float32)
    if _trace:
        return out, res
    return out



# revision 17
# speedup vs baseline: 1.0266x; 1.0266x over previous
"""Trainium2 Bass kernel for the segment-reduce masked-CE loss (nn_NewLoss).

Reference math (N=64, C=46, P=2048, MP=256):
    assignment[n, p] = 1 + (p * MP) // P  (contiguous segments of 8 frames)
    pooled[n, q, c]  = mean over the 8 frames of segment q of input[n, c, :]
    loss = -sum_{n,q} lab_mask[n,q] * log_softmax(pooled)[n, q, target[n,q]]

Sharding: data-parallel over batch n across 8 cores (8 items per core);
each core returns per-q partial sums, reduced on the host.

Per-core layout (v2): frames on partitions so the PE does the pooling.
x is shipped fp8_e4m3 as xT[frame, row] (row = 46*item + ch), 16 blocks of
128 frames. Pool matmul: lhsT = PoolSel (1/8 one-hot of u//8), rhs = x
block; fp8 DoubleRow processes two blocks per matmul (8 matmuls total),
filling two PSUM tiles S[q(128), row(368)] with the segment means.
Epilogue: EXP (scalar) -> window-46 reduce (DVE) per (q, item) = sumexp;
picked via scalar_tensor_tensor accumulate against a host-built masked
one-hot; Ln
(scalar, same act table as Exp) -> mask-weighted TTR. Host sums the
[128, 4] per-core partials.
"""

import numpy as np

import concourse.bacc as bacc
import concourse.bass as bass
import concourse.tile as tile
from concourse import mybir
from concourse.bass_utils import run_bass_kernel_spmd

F32 = mybir.dt.float32
BF16 = mybir.dt.bfloat16
F8 = mybir.dt.float8e4

N, C, P, MP = 64, 46, 2048, 256
NCORES = 8
NLOC = N // NCORES            # 8 batch items per core
ROWS = NLOC * C               # 368 (item, channel) rows per core
W = P // MP                   # 8-frame pooling window
NBLK = P // 128               # 16 frame blocks of 128
NPAIR = NBLK // 2             # 8 DoubleRow block pairs
XCOLS = NPAIR * 2 * ROWS      # 5888

# Single combined Exp+Ln activation table: drop Exp/Ln from the per-func
# tables so the fixpoint pass lands on natural_log_exp_and_others and the
# kernel pays only one ACT_TABLE_LOAD (overlapped with the input DMA).
_ORIG_GAT = bacc.get_activation_tables


def _gat_combined(arch):
    exp = mybir.ActivationFunctionType.Exp
    ln = mybir.ActivationFunctionType.Ln
    out = {}
    for name, funcs in _ORIG_GAT(arch).items():
        if name != "natural_log_exp_and_others":
            funcs = funcs - {exp, ln}
        out[name] = funcs
    return out


bacc.get_activation_tables = _gat_combined


def _build_nc():
    nc = bacc.Bacc("TRN2", target_bir_lowering=False)

    x_d = nc.dram_tensor("x", [128, XCOLS], F8, kind="ExternalInput")
    w_d = nc.dram_tensor("w", [128, 1024], F8, kind="ExternalInput")
    oh_d = nc.dram_tensor("oh", [128, 2 * ROWS], BF16, kind="ExternalInput")
    mk_d = nc.dram_tensor("mk", [128, 2 * NLOC], F32, kind="ExternalInput")
    loss_d = nc.dram_tensor("loss", [128, 4], F32, kind="ExternalOutput")

    mult = mybir.AluOpType.mult

    with tile.TileContext(nc) as tc:
        with (
            tc.tile_pool(name="xin", bufs=1) as xin,
            tc.tile_pool(name="sb", bufs=1) as sb,
            tc.tile_pool(name="psum", bufs=1, space="PSUM") as psum,
        ):
            # x chunks on the sync ring (one chunk per DoubleRow pair);
            # constants on the scalar ring in parallel.
            x8 = xin.tile([128, XCOLS], F8)
            CW = 2 * ROWS
            for b2 in range(NPAIR):
                nc.sync.dma_start(
                    out=x8[:, b2 * CW : (b2 + 1) * CW],
                    in_=x_d[:, b2 * CW : (b2 + 1) * CW],
                )
            w8 = sb.tile([128, 1024], F8)
            nc.scalar.dma_start(out=w8[:], in_=w_d[:])
            mk = sb.tile([128, 2 * NLOC], F32)
            nc.scalar.dma_start(out=mk[:], in_=mk_d[:])
            oh = sb.tile([128, 2 * ROWS], BF16)
            nc.scalar.dma_start(out=oh[:], in_=oh_d[:])

            SA = psum.tile([128, ROWS], F32, tag="SA")
            SB = psum.tile([128, ROWS], F32, tag="SB")
            # DoubleRow pooling: each matmul covers one block pair (2 k-tiles
            # of 128 frames); its 32 segments land in partition band
            # 32*(b2%4) of the full 128-partition output via the weight
            # layout (DoubleRow dst must start at partition 0), so 4 pairs
            # accumulate into one PSUM tile.
            w_vars = [
                w8[:, 256 * p : 256 * (p + 1)].rearrange("u (t m) -> u t m", t=2)
                for p in range(4)
            ]
            for b2 in range(NPAIR):
                S = SA if b2 < NPAIR // 2 else SB
                p = b2 % 4
                nc.tensor.matmul(
                    out=S[:, :],
                    lhsT=w_vars[p],
                    rhs=x8[:, b2 * CW : (b2 + 1) * CW].rearrange(
                        "u (t n) -> u t n", t=2
                    ),
                    start=(p == 0),
                    stop=(p == 3),
                    perf_mode=mybir.MatmulPerfMode.DoubleRow,
                )

            se = sb.tile([128, 2 * NLOC], F32)
            lnse = sb.tile([128, 2 * NLOC], F32)
            res = sb.tile([128, 4], F32)
            for half, S in ((0, SA), (1, SB)):
                dmp = sb.tile([128, ROWS], F32, tag=f"dmp{half}")
                nc.vector.scalar_tensor_tensor(
                    out=dmp[:],
                    in0=S[:],
                    scalar=1.0,
                    in1=oh[:, half * ROWS : (half + 1) * ROWS],
                    op0=mult,
                    op1=mult,
                    accum_out=res[:, 1 + half : 2 + half],
                )
                E = sb.tile([128, ROWS], BF16, tag=f"E{half}")
                nc.scalar.activation(
                    out=E[:], in_=S[:], func=mybir.ActivationFunctionType.Exp
                )
                nc.vector.reduce_sum(
                    out=se[:, half * NLOC : (half + 1) * NLOC],
                    in_=E[:].rearrange("q (i c) -> q i c", c=C),
                    axis=mybir.AxisListType.X,
                )
                nc.scalar.activation(
                    out=lnse[:, half * NLOC : (half + 1) * NLOC],
                    in_=se[:, half * NLOC : (half + 1) * NLOC],
                    func=mybir.ActivationFunctionType.Ln,
                )
                dms = sb.tile([128, NLOC], F32, tag=f"dms{half}")
                nc.vector.scalar_tensor_tensor(
                    out=dms[:],
                    in0=lnse[:, half * NLOC : (half + 1) * NLOC],
                    scalar=1.0,
                    in1=mk[:, half * NLOC : (half + 1) * NLOC],
                    op0=mult,
                    op1=mult,
                    accum_out=res[:, 3 * half : 3 * half + 1],
                )
            nc.sync.dma_start(out=loss_d[:], in_=res[:])

    nc.finalize()
    return nc


_NC = None


def _get_nc():
    global _NC
    if _NC is None:
        _NC = _build_nc()
    return _NC


def _make_w8():
    import ml_dtypes

    w8 = np.zeros((128, 1024), dtype=np.float32)
    u = np.arange(128)
    for p in range(4):
        for t in range(2):
            for s in range(16):
                m = 32 * p + 16 * t + s
                w8[u // 8 == s, 256 * p + 128 * t + m] = 1.0 / W
    return w8.astype(ml_dtypes.float8_e4m3fn)


_W8 = None


def make_in_maps(input, target, lab_mask):
    import ml_dtypes

    global _W8
    if _W8 is None:
        _W8 = _make_w8()
    inp = np.asarray(input)
    tgt = np.asarray(target)
    msk = np.asarray(lab_mask)
    in_maps = []
    for cc in range(NCORES):
        xl = inp[cc * NLOC : (cc + 1) * NLOC]                  # [8, 46, 2048]
        xT = xl.transpose(2, 0, 1).reshape(P, ROWS)            # [f, 46*i + c]
        x8 = np.ascontiguousarray(
            xT.reshape(NBLK, 128, ROWS).transpose(1, 0, 2).reshape(128, XCOLS)
        ).astype(ml_dtypes.float8_e4m3fn)
        tl = tgt[cc * NLOC : (cc + 1) * NLOC]                  # [8, 256]
        ml = msk[cc * NLOC : (cc + 1) * NLOC].astype(np.float32)
        ohsel = -(ml[:, :, None] * (tl[:, :, None] == np.arange(C)[None, None, :]))
        ohsel = ohsel.transpose(1, 0, 2).reshape(MP, ROWS)     # [q, 46*i + c]
        mT = ml.T                                              # [q, i]
        oh = np.ascontiguousarray(
            np.concatenate([ohsel[:128], ohsel[128:]], axis=1)
        ).astype(ml_dtypes.bfloat16)
        mk = np.ascontiguousarray(np.concatenate([mT[:128], mT[128:]], axis=1))
        in_maps.append({"x": x8, "w": _W8, "oh": oh, "mk": mk})
    return in_maps


def kernel(input, target, assignment, lab_mask, _trace=False):
    in_maps = make_in_maps(input, target, lab_mask)
    nc = _get_nc()
    res = run_bass_kernel_spmd(nc, in_maps, core_ids=list(range(NCORES)), trace=_trace)
    total = np.float64(0.0)
    for r in res.results:
        total += np.float64(r["loss"].sum())
    out = np.array(total, dtype=np.float32)
    if _trace:
        return out, res
    return out


# revision 22
# speedup vs baseline: 1.2012x; 1.1700x over previous
"""Trainium2 Bass kernel for the segment-reduce masked-CE loss (nn_NewLoss).

Reference math (N=64, C=46, P=2048, MP=256):
    assignment[n, p] = 1 + (p * MP) // P  (contiguous segments of 8 frames)
    pooled[n, q, c]  = mean over the 8 frames of segment q of input[n, c, :]
    loss = -sum_{n,q} lab_mask[n,q] * log_softmax(pooled)[n, q, target[n,q]]

Sharding: data-parallel over batch n across 8 cores (8 items per core);
each core returns per-q partial sums, reduced on the host.

Per-core layout (v2.1): frames on partitions so the PE does the pooling.
x is shipped fp8_e4m3 as xT[frame, row] (row = 46*item + ch), 16 blocks of
128 frames; one fp8 DoubleRow matmul per block pair (shared [128,2,32]
pool weights, out = 32 segments at PSUM partition base 32*(b2%3)), three
PSUM tiles S[q, row]. Input DMA is striped across four HWDGE rings
(per-ring bandwidth is the real limit, ~60-110 GB/s); weights ride a
fifth ring so the first matmul is gated only by its own data.
Epilogue per tile: picked via scalar_tensor_tensor accumulate against a
host-built masked one-hot (DVE), EXP (scalar) -> window-46 reduce (DVE)
-> Ln (scalar, same act table as Exp) -> mask STT. Host sums [128, 8].
"""

import numpy as np

import concourse.bacc as bacc
import concourse.bass as bass
import concourse.tile as tile
from concourse import mybir
from concourse.bass_utils import run_bass_kernel_spmd

F32 = mybir.dt.float32
BF16 = mybir.dt.bfloat16
F8 = mybir.dt.float8e4

N, C, P, MP = 64, 46, 2048, 256
NCORES = 8
NLOC = N // NCORES            # 8 batch items per core
ROWS = NLOC * C               # 368 (item, channel) rows per core
W = P // MP                   # 8-frame pooling window
NBLK = P // 128               # 16 frame blocks of 128
NPAIR = NBLK // 2             # 8 DoubleRow block pairs
XCOLS = NPAIR * 2 * ROWS      # 5888
CW = 2 * ROWS                 # 736 x cols per pair
NT = 3                        # PSUM tiles (32-seg bands at bases 0/32/64)
TP = (96, 96, 64)             # partitions per tile

# Single combined Exp+Ln activation table: drop Exp/Ln from the per-func
# tables so the fixpoint pass lands on natural_log_exp_and_others and the
# kernel pays only one ACT_TABLE_LOAD (overlapped with the input DMA).
_ORIG_GAT = bacc.get_activation_tables


def _gat_combined(arch):
    exp = mybir.ActivationFunctionType.Exp
    ln = mybir.ActivationFunctionType.Ln
    out = {}
    for name, funcs in _ORIG_GAT(arch).items():
        if name != "natural_log_exp_and_others":
            funcs = funcs - {exp, ln}
        out[name] = funcs
    return out


bacc.get_activation_tables = _gat_combined


def _build_nc():
    nc = bacc.Bacc("TRN2", target_bir_lowering=False)

    x_d = nc.dram_tensor("x", [128, XCOLS], F8, kind="ExternalInput")
    w_d = nc.dram_tensor("w", [128, 1024], F8, kind="ExternalInput")
    oh_d = nc.dram_tensor("oh", [128, 2 * ROWS], BF16, kind="ExternalInput")
    mk_d = nc.dram_tensor("mk", [128, 2 * NLOC], F32, kind="ExternalInput")
    loss_d = nc.dram_tensor("loss", [128, 4], F32, kind="ExternalOutput")

    mult = mybir.AluOpType.mult

    with tile.TileContext(nc) as tc:
        with (
            tc.tile_pool(name="xin", bufs=1) as xin,
            tc.tile_pool(name="sb", bufs=1) as sb,
            tc.tile_pool(name="psum", bufs=1, space="PSUM") as psum,
        ):
            # Per-ring DMA bandwidth (~60-110 GB/s) is the real limit, so
            # stripe across all three DMA-capable rings: scalar carries the
            # tiny weights first (unblocks matmul 0) then the leading x
            # pairs, sync and gpsimd carry the rest, oh/mk trail on scalar.
            w8 = sb.tile([128, 1024], F8)
            nc.scalar.dma_start(out=w8[:], in_=w_d[:])
            x8 = xin.tile([128, XCOLS], F8)
            stripes = [
                (nc.scalar, 0, 1),
                (nc.scalar, 1, 3),
                (nc.sync, 3, 6),
                (nc.gpsimd, 6, 8),
            ]
            for eng, p0, p1 in stripes:
                eng.dma_start(
                    out=x8[:, p0 * CW : p1 * CW], in_=x_d[:, p0 * CW : p1 * CW]
                )
            oh = sb.tile([128, 2 * ROWS], BF16)
            nc.scalar.dma_start(out=oh[:], in_=oh_d[:])
            mk = sb.tile([128, 2 * NLOC], F32)
            nc.scalar.dma_start(out=mk[:], in_=mk_d[:])

            res = sb.tile([128, 4], F32)

            # DoubleRow pooling: each matmul covers one block pair (2
            # k-tiles of 128 frames); its 32 segments land in partition
            # band 32*(b2%4) of the full 128-partition output via the
            # weight layout (DoubleRow dst must start at partition 0), so
            # 4 pairs accumulate into one PSUM tile.
            SA = psum.tile([128, ROWS], F32, tag="SA")
            SB = psum.tile([128, ROWS], F32, tag="SB")
            w_vars = [
                w8[:, 256 * p : 256 * (p + 1)].rearrange("u (t m) -> u t m", t=2)
                for p in range(4)
            ]
            for b2 in range(NPAIR):
                S = SA if b2 < NPAIR // 2 else SB
                p = b2 % 4
                nc.tensor.matmul(
                    out=S[:, :],
                    lhsT=w_vars[p],
                    rhs=x8[:, b2 * CW : (b2 + 1) * CW].rearrange(
                        "u (t n) -> u t n", t=2
                    ),
                    start=(p == 0),
                    stop=(p == 3),
                    perf_mode=mybir.MatmulPerfMode.DoubleRow,
                )

            se = sb.tile([128, 2 * NLOC], F32)
            lnse = sb.tile([128, 2 * NLOC], F32)
            for k, S in ((0, SA), (1, SB)):
                p = 128
                dmp = sb.tile([p, ROWS], F32, tag=f"dmp{k}")
                nc.vector.scalar_tensor_tensor(
                    out=dmp[:],
                    in0=S[:],
                    scalar=1.0,
                    in1=oh[:, k * ROWS : (k + 1) * ROWS],
                    op0=mult,
                    op1=mult,
                    accum_out=res[:, k : k + 1],
                )
                E = sb.tile([p, ROWS], BF16, tag=f"E{k}")
                nc.scalar.activation(
                    out=E[:], in_=S[:], func=mybir.ActivationFunctionType.Exp
                )
                nc.vector.reduce_sum(
                    out=se[:, k * NLOC : (k + 1) * NLOC],
                    in_=E[:].rearrange("q (i c) -> q i c", c=C),
                    axis=mybir.AxisListType.X,
                )
                nc.scalar.activation(
                    out=lnse[:, k * NLOC : (k + 1) * NLOC],
                    in_=se[:, k * NLOC : (k + 1) * NLOC],
                    func=mybir.ActivationFunctionType.Ln,
                )
                dms = sb.tile([p, NLOC], F32, tag=f"dms{k}")
                nc.vector.scalar_tensor_tensor(
                    out=dms[:],
                    in0=lnse[:, k * NLOC : (k + 1) * NLOC],
                    scalar=1.0,
                    in1=mk[:, k * NLOC : (k + 1) * NLOC],
                    op0=mult,
                    op1=mult,
                    accum_out=res[:, 2 + k : 3 + k],
                )
            nc.sync.dma_start(out=loss_d[:], in_=res[:])

    nc.finalize()
    return nc


_NC = None


def _get_nc():
    global _NC
    if _NC is None:
        _NC = _build_nc()
    return _NC


def _make_w8():
    import ml_dtypes

    w8 = np.zeros((128, 1024), dtype=np.float32)
    u = np.arange(128)
    for p in range(4):
        for t in range(2):
            for s in range(16):
                m = 32 * p + 16 * t + s
                w8[u // 8 == s, 256 * p + 128 * t + m] = 1.0 / W
    return w8.astype(ml_dtypes.float8_e4m3fn)


_W8 = None


def make_in_maps(input, target, lab_mask):
    import ml_dtypes

    global _W8
    if _W8 is None:
        _W8 = _make_w8()
    inp = np.asarray(input)
    tgt = np.asarray(target)
    msk = np.asarray(lab_mask)
    in_maps = []
    for cc in range(NCORES):
        xl = inp[cc * NLOC : (cc + 1) * NLOC]                  # [8, 46, 2048]
        xT = xl.transpose(2, 0, 1).reshape(P, ROWS)            # [f, 46*i + c]
        x8 = np.ascontiguousarray(
            xT.reshape(NBLK, 128, ROWS).transpose(1, 0, 2).reshape(128, XCOLS)
        ).astype(ml_dtypes.float8_e4m3fn)
        tl = tgt[cc * NLOC : (cc + 1) * NLOC]                  # [8, 256]
        ml = msk[cc * NLOC : (cc + 1) * NLOC].astype(np.float32)
        ohsel = -(ml[:, :, None] * (tl[:, :, None] == np.arange(C)[None, None, :]))
        ohsel = ohsel.transpose(1, 0, 2).reshape(MP, ROWS)     # [q, 46*i + c]
        mT = ml.T                                              # [q, i]
        oh = np.ascontiguousarray(
            np.concatenate([ohsel[:128], ohsel[128:]], axis=1)
        ).astype(ml_dtypes.bfloat16)
        mk = np.ascontiguousarray(np.concatenate([mT[:128], mT[128:]], axis=1))
        in_maps.append({"x": x8, "w": _W8, "oh": oh, "mk": mk})
    return in_maps


def kernel(input, target, assignment, lab_mask, _trace=False):
    in_maps = make_in_maps(input, target, lab_mask)
    nc = _get_nc()
    res = run_bass_kernel_spmd(nc, in_maps, core_ids=list(range(NCORES)), trace=_trace)
    total = np.float64(0.0)
    for r in res.results:
        total += np.float64(r["loss"].sum())
    out = np.array(total, dtype=np.float32)
    if _trace:
        return out, res
    return out
